# revision 16
# baseline (speedup 1.0000x reference)
"""Trainium2 Bass kernel for nn_EquivariantHardAlignmentModel.

8 NeuronCores, SPMD (identical program, per-core data):
  - LSTM recurrences run H-major / weight-stationary: each step streams the
    hidden state (and the gathered x embedding) through 24 stationary
    128x128 weight tiles, so gates land on full 128 partitions, no PE
    transposes are needed, and per-step PE cost is the LDWEIGHTS floor.
    enc-fwd and enc-bwd share every weight load (64 moving columns/step);
    the decoder runs the same way in a second phase.
  - The G-stack (embed/conv/logits/Z), ys gathers, bilinear alignment and
    loss tail are data-parallel: each core does 4 of 32 batch rows.  Inputs
    are batch-permuted per core so its rows are always rows 0..3 -> one
    shared program.
  - p[b,j] = log(sum_i exp(lys+eij-lnZ)) - log(sum_i exp(eij)) via
    PSUM-accumulated matmuls + ACT Exp(accum_out).  Host sums & negates.
Phase order: LSTM-A (fwd+bwd) -> G -> LSTM-B (dec) -> final, so the PE
never waits on the gpsimd gathers that feed G.
"""

import os
import sys

sys.path.insert(0, "/opt/trn_rl_repo")

import numpy as np
import ml_dtypes

import concourse.bass as bass
import concourse.mybir as mybir
import concourse.tile as tile
from concourse import bacc
from concourse.bass_utils import run_bass_kernel_spmd
from concourse.masks import make_identity

BF = mybir.dt.bfloat16
F32 = mybir.dt.float32
AF = mybir.ActivationFunctionType

B, NE, ND = 32, 512, 512
V = 2000
H, F, KW, PG = 256, 256, 5, 4
EE, ED = 128, 128
NCORES, BPC = 8, 4
XCH = 4096  # columns per x-gather chunk tile (128 steps * 32 batch)

# gate -> (n-tile pair) in PyTorch i,f,g,o row order
GATE_NT = (("g", (4, 5)), ("f", (2, 3)), ("i", (0, 1)), ("o", (6, 7)))


def _bf(x):
    return np.ascontiguousarray(x.astype(ml_dtypes.bfloat16))


def _wrap16(flat):
    """index list -> (128, n/16) int16, dma_gather wrapped + 8x replicated."""
    flat = np.asarray(flat).reshape(-1)
    assert flat.size % 16 == 0
    w = flat.reshape(-1, 16).T.astype(np.int16)  # (16, n/16)
    return np.ascontiguousarray(np.tile(w, (8, 1)))


# ---------------------------------------------------------------------------
# device program
# ---------------------------------------------------------------------------

def build_program(n_enc=NE, n_dec=ND):
    from contextlib import ExitStack

    nc = bacc.Bacc(None, target_bir_lowering=False, debug=False)
    xch = min(XCH, n_enc * B)  # columns per x chunk tile
    n_xc = n_enc * B // xch  # x chunk tiles per sequence
    n_yc = n_dec * B // xch

    with tile.TileContext(nc) as tc, ExitStack() as es:
        dram = es.enter_context(tc.tile_pool(name="dram", bufs=1, space="DRAM"))

        def din(name, shape, dtype):
            return dram.tile(shape, dtype, kind="ExternalInput", name=name,
                             uniquify=False)

        x_enc_idx = din("x_enc_idx", [128, B * n_enc // 16], mybir.dt.int16)
        y_dec_idx = din("y_dec_idx", [128, B * n_dec // 16], mybir.dt.int16)
        e_idx = din("e_idx", [128, BPC * NE // 16], mybir.dt.int16)
        gb_idx = din("gb_idx", [128, BPC * ND // 16], mybir.dt.int16)
        gembed_bf = din("gembed_bf", [V, F], BF)
        enc_embed_bf = din("enc_embed_bf", [V, EE], BF)
        dec_embed_bf = din("dec_embed_bf", [V, ED], BF)
        w2t_bf = din("w2t_bf", [V, F], BF)
        w2_d = din("w2_d", [128, 2, V], BF)
        gconv_d = din("gconv_d", [128, KW * 4, 128], BF)
        # H-major weight tiles: wih [128E, nt, 128n]; whh [128k, nt*2+kc, 128n]
        wih_e_d = din("wih_e_d", [128, 8, 128], BF)
        whh_e_d = din("whh_e_d", [128, 16, 128], BF)
        wih_d_d = din("wih_d_d", [128, 8, 128], BF)
        whh_d_d = din("whh_d_d", [128, 16, 128], BF)
        tt_d = din("tt_d", [128, 8, 128], BF)
        pout = dram.tile([128, 16], F32, kind="ExternalOutput", name="pout",
                         uniquify=False)

        cpool = es.enter_context(tc.tile_pool(name="const", bufs=1))

        idf32 = cpool.tile([128, 128], F32)
        make_identity(nc, idf32[:])
        negones = cpool.tile([1, 128], F32)
        nc.gpsimd.memset(negones[:], -1.0)

        def to_sbuf(ap, name):
            t = cpool.tile(list(ap.shape), ap.dtype, name=name)
            nc.sync.dma_start(out=t[:], in_=ap[:])
            return t

        w2_sb = to_sbuf(w2_d, "w2_sb")
        gconv_sb = to_sbuf(gconv_d, "gconv_sb")
        wih_e = to_sbuf(wih_e_d, "wih_e")
        whh_e = to_sbuf(whh_e_d, "whh_e")
        wih_dd = to_sbuf(wih_d_d, "wih_dd")
        whh_dd = to_sbuf(whh_d_d, "whh_dd")
        tt_sb = to_sbuf(tt_d, "tt_sb")
        xidx_sb = to_sbuf(x_enc_idx, "xidx_sb")
        yidx_sb = to_sbuf(y_dec_idx, "yidx_sb")
        eidx_sb = to_sbuf(e_idx, "eidx_sb")
        gbidx_sb = to_sbuf(gb_idx, "gbidx_sb")

        # zero LSTM init state: must hit the gpsimd queue BEFORE the big
        # gathers, or phase A's first step waits ~500us behind them
        hc0 = cpool.tile([128, 2, 64], BF, name="hc0")
        cc0 = cpool.tile([128, 2, 64], BF, name="cc0")
        nc.gpsimd.memset(hc0[:], 0.0)
        nc.gpsimd.memset(cc0[:], 0.0)

        gpool = es.enter_context(tc.tile_pool(name="gath", bufs=1))

        def chunk_gather(table, idx_sb, nchunks, name):
            tiles = []
            for k in range(nchunks):
                t = gpool.tile([128, 1, xch], BF, name=f"{name}{k}")
                tiles.append(t)
            return tiles

        xgc = chunk_gather(enc_embed_bf, xidx_sb, n_xc, "xg")
        ygc = chunk_gather(dec_embed_bf, yidx_sb, n_yc, "yg")

        def issue_gather(tiles, table, idx_sb, order):
            for k in order:
                nc.gpsimd.dma_gather(
                    out_ap=tiles[k][:, :, :], in_ap=table[:],
                    idxs_ap=idx_sb[:, k * xch // 16:(k + 1) * xch // 16],
                    num_idxs=xch, num_idxs_reg=xch, elem_size=EE,
                    transpose=True, single_packet=False)

        # fwd needs chunk 0 first, bwd needs the last chunk first
        xorder = list(range(n_xc))
        if n_xc > 1:
            xorder = [xorder[0], xorder[-1]] + xorder[1:-1]
        issue_gather(xgc, enc_embed_bf, xidx_sb, xorder)

        eT = [gpool.tile([128, 2, NE], BF, name=f"eT{b}") for b in range(BPC)]
        gbT = [gpool.tile([128, 2, ND], BF, name=f"gbT{b}") for b in range(BPC)]
        for b in range(BPC):
            nc.gpsimd.dma_gather(
                out_ap=eT[b][:], in_ap=gembed_bf[:],
                idxs_ap=eidx_sb[:, b * NE // 16:(b + 1) * NE // 16],
                num_idxs=NE, num_idxs_reg=NE, elem_size=F, transpose=True)
            nc.gpsimd.dma_gather(
                out_ap=gbT[b][:], in_ap=w2t_bf[:],
                idxs_ap=gbidx_sb[:, b * ND // 16:(b + 1) * ND // 16],
                num_idxs=ND, num_idxs_reg=ND, elem_size=F, transpose=True)

        issue_gather(ygc, dec_embed_bf, yidx_sb, list(range(n_yc)))

        # persistent activation stores
        spool = es.enter_context(tc.tile_pool(name="stores", bufs=1))
        tcT = [spool.tile([128, 2, NE], BF, name=f"tcT{b}") for b in range(BPC)]
        lnZ = [spool.tile([1, NE], F32, name=f"lnZ{b}") for b in range(BPC)]
        hencTf = spool.tile([128, 2, BPC * NE], BF)
        hencTb = spool.tile([128, 2, BPC * NE], BF)
        hdecT = spool.tile([128, 2, BPC * (ND + 1)], BF)
        pout_sb = spool.tile([128, 16], F32)
        # t-major per-step h stores (contiguous writes); reshuffled to the
        # b-major layouts above just before the final phase
        hencFt = spool.tile([128, NE, 2, BPC], BF)
        hencBt = spool.tile([128, NE, 2, BPC], BF)
        hdecTt = spool.tile([128, ND, 2, BPC], BF)

        # ------------------------------------------------------------------
        # LSTM phase: H-major, weight-stationary.
        # PSUM banks (2KB each, padded): pg = g gate (rows 0:2), pfi = f+i
        # (rows 0:4), po = o (rows 0:2).  The x-part matmuls of step t+1 are
        # issued right after step t's h-matmuls so the PE stays busy during
        # the serial ACT/DVE tail.
        # ------------------------------------------------------------------
        lstm_sb = es.enter_context(tc.tile_pool(name="lstm_sb", bufs=2))

        BANK_NTS = (("g", ((0, 4), (1, 5))),
                    ("fi", ((0, 2), (1, 3), (2, 0), (3, 1))),
                    ("o", ((0, 6), (1, 7))))

        def lstm_phase(psp, W, n_steps, h0, ctg0, whh_sb, wih_sb,
                       x_slices_of, store_fn):
            PR = 2048 // (W * 4)

            def alloc_ps():
                return {bank: psp.tile([128, PR, W], F32, tag=f"p{bank}{W}",
                                       name=f"p{bank}")
                        for bank, _ in BANK_NTS}

            def x_mms(ps, t):
                for bank, rnts in BANK_NTS:
                    first = True
                    for row, nt in rnts:
                        for xt, c0, off, w in x_slices_of(t):
                            nc.tensor.matmul(
                                ps[bank][:, row, off:off + w],
                                wih_sb[:, nt, :], xt[:, 0, c0:c0 + w],
                                start=first, stop=False,
                                skip_group_check=True)
                            first = False

            def h_mms(ps, bank, rnts, h_prev, kc, last):
                for row, nt in rnts:
                    nc.tensor.matmul(ps[bank][:, row, 0:W],
                                     whh_sb[:, nt * 2 + kc, :],
                                     h_prev[:, kc, 0:W],
                                     start=False, stop=last,
                                     skip_group_check=True)

            cur = alloc_ps()
            x_mms(cur, 0)
            h, ctg = h0, ctg0
            for t in range(n_steps):
                # h matmuls per bank, kc0 before kc1 (kc-split h lets the
                # next step's kc0 matmuls start before h_mul1 finishes)
                for bank, rnts in BANK_NTS:
                    h_mms(cur, bank, rnts, h, 0, False)
                    h_mms(cur, bank, rnts, h, 1, True)
                    if bank == "g":
                        # tanh(g) lands in the CARRIED ctg tile rows 2:4
                        # (rows 0:2 hold c from last step) so m01 is a
                        # single fused multiply
                        nc.scalar.activation(ctg[:, 2:4, :],
                                             cur["g"][:, 0:2, :], AF.Tanh)
                    elif bank == "fi":
                        sfi = lstm_sb.tile([128, 4, W], BF, tag=f"sfi{W}")
                        nc.scalar.activation(sfi[:], cur["fi"][:, 0:4, :],
                                             AF.Sigmoid)
                # m01 = [sf*c | si*tg] in one op
                m01 = lstm_sb.tile([128, 4, W], BF, tag=f"m01{W}")
                nc.vector.tensor_mul(m01[:], sfi[:], ctg[:])
                ctg_n = lstm_sb.tile([128, 4, W], BF, tag=f"ctg{W}")
                nc.vector.tensor_add(ctg_n[:, 0, :], m01[:, 0, :],
                                     m01[:, 2, :])
                nc.vector.tensor_add(ctg_n[:, 1, :], m01[:, 1, :],
                                     m01[:, 3, :])
                so = lstm_sb.tile([128, 2, W], BF, tag=f"so{W}")
                nc.scalar.activation(so[:], cur["o"][:, 0:2, :], AF.Sigmoid)
                tc_ = lstm_sb.tile([128, 2, W], BF, tag=f"tc{W}")
                nc.scalar.activation(tc_[:, 0, :], ctg_n[:, 0, :], AF.Tanh)
                nc.scalar.activation(tc_[:, 1, :], ctg_n[:, 1, :], AF.Tanh)
                h_new = lstm_sb.tile([128, 2, W], BF, tag=f"h{W}", bufs=3)
                nc.vector.tensor_mul(h_new[:, 0, :], so[:, 0, :],
                                     tc_[:, 0, :])
                nc.vector.tensor_mul(h_new[:, 1, :], so[:, 1, :],
                                     tc_[:, 1, :])
                store_fn(t, h_new)
                if t + 1 < n_steps:
                    nxt = alloc_ps()
                    x_mms(nxt, t + 1)
                    cur = nxt
                h, ctg = h_new, ctg_n
            return h, ctg

        # ------------------------------------------------------------------
        # Phase A: enc fwd + enc bwd, fused 64 moving columns
        # ------------------------------------------------------------------
        def xs_A(t):
            tb = n_enc - 1 - t
            return [(xgc[(t * B) // xch], (t * B) % xch, 0, 32),
                    (xgc[(tb * B) // xch], (tb * B) % xch, 32, 32)]

        def store_A(t, h):
            tb = n_enc - 1 - t
            nc.vector.tensor_copy(hencFt[:, t, :, :], h[:, :, 0:BPC])
            nc.vector.tensor_copy(hencBt[:, tb, :, :], h[:, :, 32:32 + BPC])

        with tc.tile_pool(name="psA", bufs=2, space="PSUM") as psA:
            ctg0 = lstm_sb.tile([128, 4, 64], BF, tag="ctg64")
            nc.vector.tensor_copy(ctg0[:, 0:2, :], cc0[:])
            h_fin, ctg_fin = lstm_phase(psA, 64, n_enc, hc0, ctg0,
                                        whh_e, wih_e, xs_A, store_A)
            nc.vector.tensor_copy(hdecT[:, :, 0::ND + 1][:, :, 0:BPC],
                                  h_fin[:, :, 0:BPC])

        # ------------------------------------------------------------------
        # Phase G (between the LSTM phases; its gathers ran during A)
        # ------------------------------------------------------------------
        with tc.tile_pool(name="gwork", bufs=2) as gw, \
             tc.tile_pool(name="gpsum", bufs=2, space="PSUM") as gp, \
             tc.tile_pool(name="zrow", bufs=4, space="PSUM") as zrp:
            # e = tanh(gembed[xs]) — emitted here (not at gather time) so it
            # does not block phase A's ACT stream behind the gpsimd gathers
            etan = [gpool.tile([128, 2, NE], BF, name=f"etan{b}")
                    for b in range(BPC)]
            for b in range(BPC):
                nc.scalar.activation(etan[b][:], eT[b][:], AF.Tanh)
            # conv + tanh
            for b in range(BPC):
                for fo in range(2):
                    cp = gp.tile([128, NE], F32, tag="convps")
                    first = True
                    for k in [2, 0, 1, 3, 4]:
                        d = k - 2
                        lo_out, lo_in = max(0, -d), max(0, d)
                        L = NE - abs(d)
                        for fi in range(2):
                            nc.tensor.matmul(
                                cp[:, lo_out:lo_out + L],
                                gconv_sb[:, (k * 2 + fi) * 2 + fo, :],
                                etan[b][:, fi, lo_in:lo_in + L],
                                start=first, stop=(k == 4 and fi == 1),
                                skip_group_check=True)
                            first = False
                    nc.scalar.activation(tcT[b][:, fo, :], cp[:], AF.Tanh)
            # logits (t-major) -> exp -> Z
            zrows = []
            for b in range(BPC):
                zrow = zrp.tile([1, NE], F32, tag="zrow", name=f"zr{b}")
                for ic in range(4):
                    zp = gw.tile([128, 4], F32, tag="zp")
                    for vc in range(4):
                        lp = gp.tile([128, 500], F32, tag="logps")
                        for f in range(2):
                            nc.tensor.matmul(
                                lp[:], tcT[b][:, f, ic * 128:(ic + 1) * 128],
                                w2_sb[:, f, vc * 500:(vc + 1) * 500],
                                start=(f == 0), stop=(f == 1))
                        sc = gw.tile([128, 500], BF, tag="expsc")
                        nc.scalar.activation(sc[:], lp[:], AF.Exp,
                                             accum_out=zp[:, vc:vc + 1])
                    zc = gw.tile([128, 1], F32, tag="zc")
                    nc.vector.tensor_reduce(zc[:], zp[:],
                                            axis=mybir.AxisListType.X,
                                            op=mybir.AluOpType.add)
                    nc.tensor.transpose(zrow[:, ic * 128:(ic + 1) * 128],
                                        zc[:], idf32[:])
                zrows.append(zrow)
            for b in range(BPC):
                nc.scalar.activation(lnZ[b][:], zrows[b][:], AF.Ln)

        # ------------------------------------------------------------------
        # Phase B: decoder
        # ------------------------------------------------------------------
        def xs_B(t):
            return [(ygc[(t * B) // xch], (t * B) % xch, 0, 32)]

        def store_B(t, h):
            nc.vector.tensor_copy(hdecTt[:, t, :, :], h[:, :, 0:BPC])

        with tc.tile_pool(name="psB", bufs=2, space="PSUM") as psB:
            ctg0B = lstm_sb.tile([128, 4, 32], BF, tag="ctg32")
            nc.vector.tensor_copy(ctg0B[:, 0:2, :], ctg_fin[:, 0:2, 0:32])
            lstm_phase(psB, 32, n_dec, h_fin, ctg0B,
                       whh_dd, wih_dd, xs_B, store_B)

        # ------------------------------------------------------------------
        # Final phase
        # ------------------------------------------------------------------
        # reshuffle the t-major step stores into b-major contiguous layouts
        for hc in range(2):
            for b in range(BPC):
                nc.vector.tensor_copy(hencTf[:, hc, b * NE:(b + 1) * NE],
                                      hencFt[:, :, hc, b])
                nc.vector.tensor_copy(hencTb[:, hc, b * NE:(b + 1) * NE],
                                      hencBt[:, :, hc, b])
                o = b * (ND + 1) + 1
                nc.vector.tensor_copy(hdecT[:, hc, o:o + ND],
                                      hdecTt[:, :, hc, b])

        with tc.tile_pool(name="fin_sb", bufs=2) as fsb, \
             tc.tile_pool(name="fin_keep", bufs=1) as fkeep, \
             tc.tile_pool(name="fin_ps", bufs=2, space="PSUM") as fps:
            sda = [fkeep.tile([128, 8], F32, name=f"sda{b}")
                   for b in range(BPC)]
            for b in range(BPC):
                thT = fsb.tile([128, 2, NE], BF, tag="thT")
                for hc in range(2):
                    tp = fps.tile([128, NE], F32, tag="thps")
                    for ec in range(4):
                        src = hencTf if ec < 2 else hencTb
                        nc.tensor.matmul(
                            tp[:], tt_sb[:, ec * 2 + hc, :],
                            src[:, ec % 2, b * NE:(b + 1) * NE],
                            start=(ec == 0), stop=(ec == 3))
                    nc.scalar.activation(thT[:, hc, :], tp[:], AF.Copy)
                for jc in range(4):
                    fp = fps.tile([128, NE], F32, tag="fps")
                    for hc in range(2):
                        nc.tensor.matmul(
                            fp[:],
                            hdecT[:, hc, :][:, b * (ND + 1) + jc * 128:
                                            b * (ND + 1) + jc * 128 + 128],
                            thT[:, hc, :], start=(hc == 0), stop=False,
                            skip_group_check=True)
                    sc1 = fsb.tile([128, NE], BF, tag="fexp")
                    nc.scalar.activation(
                        sc1[:], fp[:], AF.Exp,
                        accum_out=sda[b][:, 2 * jc:2 * jc + 1])
                    for f in range(2):
                        nc.tensor.matmul(
                            fp[:], gbT[b][:, f, jc * 128:jc * 128 + 128],
                            tcT[b][:, f, :], start=False, stop=False,
                            skip_group_check=True)
                    nc.tensor.matmul(fp[:], negones[:, 0:128], lnZ[b][:],
                                     start=False, stop=True,
                                     skip_group_check=True)
                    sc2 = fsb.tile([128, NE], BF, tag="fexp")
                    nc.scalar.activation(
                        sc2[:], fp[:], AF.Exp,
                        accum_out=sda[b][:, 2 * jc + 1:2 * jc + 2])
            for b in range(BPC):
                lns = fsb.tile([128, 8], F32, tag="lns")
                nc.scalar.activation(lns[:], sda[b][:], AF.Ln)
                for jc in range(4):
                    nc.vector.tensor_sub(
                        pout_sb[:, b * 4 + jc:b * 4 + jc + 1],
                        lns[:, 2 * jc + 1:2 * jc + 2],
                        lns[:, 2 * jc:2 * jc + 1])
            nc.sync.dma_start(out=pout[:], in_=pout_sb[:])

    nc.compile()
    return nc


# ---------------------------------------------------------------------------
# host side
# ---------------------------------------------------------------------------

_CACHE = {}


def _get_program(n_enc, n_dec):
    key = (n_enc, n_dec)
    if key not in _CACHE:
        _CACHE[key] = build_program(n_enc, n_dec)
    return _CACHE[key]


def _host_prep(inputs, n_enc=NE, n_dec=ND):
    xs = np.asarray(inputs["xs_idx"]).astype(np.int64)
    ys = np.asarray(inputs["ys_idx"]).astype(np.int64)
    gembed_W = np.asarray(inputs["gembed_W"], np.float32)
    gconv_W = np.asarray(inputs["gconv_W"], np.float32)
    gdecode_W = np.asarray(inputs["gdecode_W"], np.float32)
    enc_embed = np.asarray(inputs["enc_embed"], np.float32)
    dec_embed = np.asarray(inputs["dec_embed"], np.float32)
    T = np.asarray(inputs["T"], np.float32)

    for nm in ("enc_b", "dec_b"):
        assert not np.any(np.asarray(inputs[nm])), f"{nm} nonzero unsupported"

    def lstm_w(wih, whh):
        wih = np.asarray(wih, np.float32)  # (4H, E)
        whh = np.asarray(whh, np.float32)  # (4H, H)
        wih_t = _bf(wih.T.reshape(128, 8, 128))
        whh_t = _bf(whh.T.reshape(2, 128, 8, 128)
                    .transpose(1, 2, 0, 3).reshape(128, 16, 128))
        return wih_t, whh_t

    wih_e_d, whh_e_d = lstm_w(inputs["enc_Wih"], inputs["enc_Whh"])
    wih_d_d, whh_d_d = lstm_w(inputs["dec_Wih"], inputs["dec_Whh"])

    w2_d = _bf(gdecode_W.reshape(2, 128, V).transpose(1, 0, 2))
    g = gconv_W.reshape(KW, 2, 128, 2, 128)
    gconv_d = _bf(np.ascontiguousarray(
        g.transpose(2, 0, 1, 3, 4).reshape(128, KW * 4, 128)))
    tt = T.T.reshape(4, 128, 2, 128)  # [ec, p, hc, c]
    tt_d = _bf(np.ascontiguousarray(
        tt.transpose(1, 0, 2, 3).reshape(128, 8, 128)))

    base = dict(
        gembed_bf=_bf(gembed_W), enc_embed_bf=_bf(enc_embed),
        dec_embed_bf=_bf(dec_embed), w2t_bf=_bf(gdecode_W.T),
        w2_d=w2_d, gconv_d=gconv_d,
        wih_e_d=wih_e_d, whh_e_d=whh_e_d,
        wih_d_d=wih_d_d, whh_d_d=whh_d_d, tt_d=tt_d,
    )

    in_maps = []
    for m in range(NCORES):
        order = np.concatenate(
            [np.arange(4 * m, 4 * m + 4),
             np.delete(np.arange(B), np.s_[4 * m:4 * m + 4])])
        xs_p, ys_p = xs[order], ys[order]
        xm = np.where(xs_p < PG, 0, xs_p)
        ym = np.where(ys_p < PG, 0, ys_p)
        im = dict(base)
        im["x_enc_idx"] = _wrap16(xm[:, :n_enc].T)   # (t,b) order
        im["y_dec_idx"] = _wrap16(ym[:, :n_dec].T)
        im["e_idx"] = _wrap16(xs_p[:BPC])            # (b,t) order
        im["gb_idx"] = _wrap16(ys_p[:BPC])
        in_maps.append(im)
    return in_maps


def kernel(**inputs):
    trace = bool(int(os.environ.get("KERNEL_TRACE", "0")))
    n_enc = int(os.environ.get("KERNEL_NENC", NE))
    n_dec = int(os.environ.get("KERNEL_NDEC", ND))
    nc = _get_program(n_enc, n_dec)
    in_maps = _host_prep(inputs, n_enc, n_dec)
    res = run_bass_kernel_spmd(nc, in_maps, list(range(NCORES)), trace=trace)
    total = np.float64(0.0)
    for r in res.results:
        total += np.asarray(r["pout"], np.float64).sum()
    kernel.last_results = res
    return np.float32(-total)
